# revision 1
# baseline (speedup 1.0000x reference)
"""Trainium2 Bass kernel for the trajectory-decoder LSTM problem.

Math (mirrors the reference, with algebraic folds):
  dec_inp feeds gates only through W_emb; W_sp/W_emb/W_hp collapse:
    W_es = W_emb @ W_sp            [4H, 2]
    gates_t = zx@W_zx.T + bias + r_{t-1}@W_es.T + h_{t-1}@W_hh.T
  For t>=1, r_{t-1} = h_{t-1}@W_hp.T + b_hp, so with
    W_hh' = W_hh + W_es @ W_hp,  bias1 = b_ih + b_hh + W_emb@b_sp + W_es@b_hp
  every step becomes uniform:  gates_t = zx@W_zx.T + bias1 + h_{t-1}@W_hh'.T
  plus a rank-2 step-0 correction (lpr - r_init)@W_es.T injected once.
  `last_pos` is dead code (output is just the stacked rel_pos).

Device strategy (pure data-parallel over 8 cores, 4096 batch each):
  - batch on the free dim, features on partitions
  - per 256-batch wave, the 4 gate pre-activations live RESIDENT in one
    2-bank PSUM tile; each step the PE accumulates (h_t - h_{t-1}) @ W_hh'.T
    into it (start=False), so no per-step zxp re-add on the vector engine.
    A K=2 "bank-open" matmul (bias x 0/1 indicator) clears has_written for
    the whole bank and injects the per-gate bias.
  - the g-gate weights are doubled on host so ONE sigmoid over all 4 banks
    yields sig(i),sig(f),sig(o),sig(2g); tanh(g)=2*sig(2g)-1 folds into the
    DVE scalar_tensor_tensor ops for m1 and c.
  - rel_pos matmuls are batched per wave over a contiguous h history tile.
  - waves are software-pipelined: emission follows a staggered round
    schedule (~2.4 chains in flight) because Tile executes each engine's
    stream in emission order.
  - all matmul operands fp16 (full PE rate), PSUM accumulation fp32;
    c kept in fp16, b_hp added on host after gather.
"""

import os
import numpy as np

B = 32768
NCORES = 8
BC = B // NCORES          # 4096 batch per core
WAVE = 256                # batch per recurrence chain (2 PSUM banks of gates)
NW = BC // WAVE           # 16 waves
PAIR = 2 * WAVE           # phase-A (mlp) runs at N=512 across wave pairs
T = 12                    # decode steps
H = 128
G4 = 4 * H                # 512 gate features
ZX = 1056
KP = 1152                 # ZX padded to 9*128
KT = KP // 128            # 9 contraction tiles
MLP = 1024
EMB = 64

_cache = {}


def _build_nc():
    import concourse.bass as bass
    import concourse.bacc as bacc
    import concourse.mybir as mybir
    import concourse.tile as tile
    from concourse.bass import ts

    f16 = mybir.dt.float16
    f32 = mybir.dt.float32
    AF = mybir.ActivationFunctionType
    OP = mybir.AluOpType

    nc = bacc.Bacc("TRN2", target_bir_lowering=False)

    zxT = nc.dram_tensor("zxT", [KP, BC], f16, kind="ExternalInput")
    lprT = nc.dram_tensor("lprT", [2, BC], f16, kind="ExternalInput")
    w1t = nc.dram_tensor("w1t", [128, KT, MLP], f16, kind="ExternalInput")
    wzxt = nc.dram_tensor("wzxt", [128, KT, G4], f16, kind="ExternalInput")
    w2t = nc.dram_tensor("w2t", [128, 8, H], f16, kind="ExternalInput")
    whht = nc.dram_tensor("whht", [128, G4], f16, kind="ExternalInput")
    whpt = nc.dram_tensor("whpt", [128, 2], f16, kind="ExternalInput")
    k3 = nc.dram_tensor("k3", [2, G4], f16, kind="ExternalInput")   # -W_es.T
    wes = nc.dram_tensor("wes", [2, G4], f16, kind="ExternalInput")  # +W_es.T
    bias2 = nc.dram_tensor("bias2", [2, 2 * 128], f16, kind="ExternalInput")
    ind = nc.dram_tensor("ind", [2, 2 * WAVE], f16, kind="ExternalInput")
    b1 = nc.dram_tensor("b1", [128, 8], f32, kind="ExternalInput")
    b2 = nc.dram_tensor("b2", [128, 1], f32, kind="ExternalInput")
    bhp = nc.dram_tensor("bhp", [2, 1], f32, kind="ExternalInput")
    pred = nc.dram_tensor("pred", [T, 2, BC], f32, kind="ExternalOutput")

    with tile.TileContext(nc) as tc:
        with (
            tc.tile_pool(name="consts", bufs=1) as cpool,
            tc.tile_pool(name="zx", bufs=2) as zxpool,
            tc.tile_pool(name="h1", bufs=2) as h1pool,
            tc.tile_pool(name="hc", bufs=10) as hcpool,
            tc.tile_pool(name="acts", bufs=6) as apool,
            tc.tile_pool(name="outs", bufs=3) as opool,
            tc.tile_pool(name="scrps", bufs=2, space="PSUM") as scrpool,
            tc.tile_pool(name="gateps", bufs=3, space="PSUM") as gatepool,
        ):
            # ---- load constants once ----
            w1t_s = cpool.tile([128, KT, MLP], f16)
            nc.sync.dma_start(w1t_s[:], w1t[:])
            wzxt_s = cpool.tile([128, KT, G4], f16)
            nc.sync.dma_start(wzxt_s[:], wzxt[:])
            w2t_s = cpool.tile([128, 8, H], f16)
            nc.sync.dma_start(w2t_s[:], w2t[:])
            whht_s = cpool.tile([128, G4], f16)
            nc.sync.dma_start(whht_s[:], whht[:])
            whpt_s = cpool.tile([128, 2], f16)
            nc.sync.dma_start(whpt_s[:], whpt[:])
            k3_s = cpool.tile([2, G4], f16)
            nc.sync.dma_start(k3_s[:], k3[:])
            wes_s = cpool.tile([2, G4], f16)
            nc.sync.dma_start(wes_s[:], wes[:])
            bias2_s = cpool.tile([2, 2 * 128], f16)
            nc.sync.dma_start(bias2_s[:], bias2[:])
            ind_s = cpool.tile([2, 2 * WAVE], f16)
            nc.sync.dma_start(ind_s[:], ind[:])
            b1_s = cpool.tile([128, 8], f32)
            nc.sync.dma_start(b1_s[:], b1[:])
            b2_s = cpool.tile([128, 1], f32)
            nc.sync.dma_start(b2_s[:], b2[:])
            bhp_s = cpool.tile([2, 1], f32)
            nc.sync.dma_start(bhp_s[:], bhp[:])
            lpr_s = cpool.tile([2, BC], f16)
            nc.sync.dma_start(lpr_s[:], lprT[:])

            zxT_v = zxT.rearrange("(k p) b -> p k b", p=128)
            pred_v = pred.rearrange("t j b -> j t b")

            # Staggered-chain schedule: wave w's steps run in rounds
            # [5w+2, 5w+14); ~2.4 chains active per round so ACT/DVE/GPS
            # stay busy. Emission order IS execution order per engine.
            NP = NW // 2
            state = [dict() for _ in range(NW)]
            pair_state = [dict() for _ in range(NP)]
            events = []  # (round, prio, fn)

            def mk_mlp1(p, j):
                def fn():
                    st = pair_state[p]
                    if "zxw" not in st:
                        zxw = zxpool.tile([128, KT, PAIR], f16, tag="zxw", name="zxw")
                        nc.sync.dma_start(zxw[:], zxT_v[:, :, ts(p, PAIR)])
                        st["zxw"] = zxw
                        st["h1"] = h1pool.tile([128, 8, PAIR], f16, tag="h1", name="h1")
                    ps = scrpool.tile([128, PAIR], f32, tag="scratch", name="ps")
                    for k in range(KT):
                        nc.tensor.matmul(
                            ps[:], w1t_s[:, k, ts(j, 128)], st["zxw"][:, k, :],
                            start=(k == 0), stop=(k == KT - 1),
                        )
                    nc.vector.tensor_scalar(
                        st["h1"][:, j, :], ps[:], b1_s[:, j : j + 1], 0.0,
                        OP.add, OP.max,
                    )
                return fn

            def mk_mlp2(p):
                def fn():
                    st = pair_state[p]
                    ps = scrpool.tile([128, PAIR], f32, tag="scratch", name="ps")
                    for j in range(8):
                        nc.tensor.matmul(
                            ps[:], w2t_s[:, j, :], st["h1"][:, j, :],
                            start=(j == 0), stop=(j == 7),
                        )
                    hi = h1pool.tile([128, PAIR], f16, tag="hinit", name="hinit")
                    nc.vector.tensor_scalar(
                        hi[:], ps[:], b2_s[:, 0:1], 0.0, OP.add, OP.max
                    )
                    st["h_init"] = hi
                return fn

            def mk_init1(w):
                def fn():
                    st = state[w]
                    pst = pair_state[w // 2]
                    hs = ts(w % 2, WAVE)
                    st["h_prev"] = pst["h_init"][:, hs]
                    gates = gatepool.tile([128, 4 * WAVE], f32, tag="gates", name="gates")
                    st["gates"] = gates
                    for bk in range(2):
                        nc.tensor.matmul(
                            gates[:, ts(bk, 2 * WAVE)], bias2_s[:, ts(bk, 128)],
                            ind_s[:], start=True, stop=False, skip_group_check=True,
                        )
                    for g in range(2):
                        gp = gates[:, ts(g, WAVE)]
                        for k in range(KT):
                            nc.tensor.matmul(
                                gp[:], wzxt_s[:, k, ts(g, 128)],
                                pst["zxw"][:, k, hs],
                                start=False, stop=False, skip_group_check=True,
                            )
                return fn

            def mk_init2(w):
                def fn():
                    st = state[w]
                    pst = pair_state[w // 2]
                    hs = ts(w % 2, WAVE)
                    gates = st["gates"]
                    for g in range(2, 4):
                        gp = gates[:, ts(g, WAVE)]
                        for k in range(KT):
                            nc.tensor.matmul(
                                gp[:], wzxt_s[:, k, ts(g, 128)],
                                pst["zxw"][:, k, hs],
                                start=False, stop=False, skip_group_check=True,
                            )
                    psr = scrpool.tile([2, WAVE], f32, tag="scratch", name="psr")
                    nc.tensor.matmul(
                        psr[:], whpt_s[:], st["h_prev"][:], start=True, stop=True
                    )
                    k3rhs = apool.tile([2, WAVE], f16, tag="k3rhs", name="k3rhs")
                    nc.vector.scalar_tensor_tensor(
                        k3rhs[:], psr[:], bhp_s[:, 0:1], lpr_s[:, ts(w, WAVE)],
                        OP.add, OP.subtract,
                    )
                    st["k3rhs"] = k3rhs
                    for g in range(4):
                        gp = gates[:, ts(g, WAVE)]
                        nc.tensor.matmul(
                            gp[:], whht_s[:, ts(g, 128)], st["h_prev"][:],
                            start=False, stop=False, skip_group_check=True,
                        )
                        nc.tensor.matmul(
                            gp[:], k3_s[:, ts(g, 128)], k3rhs[:],
                            start=False, stop=False, skip_group_check=True,
                        )
                    st["predsb"] = opool.tile([2, T * WAVE], f32, tag="predsb", name="predsb")
                return fn

            def mk_s1(w, t):
                def fn():
                    st = state[w]
                    gates = st["gates"]
                    sig = apool.tile([128, 4 * WAVE], f16, tag="sig", name="sig")
                    nc.scalar.activation(sig[:], gates[:], AF.Sigmoid)
                    # m1' = (sig(2g) - 0.5) * sig(i)  ==  (i_act * tanh_g)/2
                    m1 = apool.tile([128, WAVE], f16, tag="m1", name="m1")
                    nc.vector.scalar_tensor_tensor(
                        m1[:], sig[:, 3 * WAVE : 4 * WAVE], 0.5, sig[:, 0:WAVE],
                        OP.subtract, OP.mult,
                    )
                    if t > 0:
                        m2 = apool.tile([128, WAVE], f16, tag="m2", name="m2")
                        nc.gpsimd.tensor_tensor(
                            m2[:], sig[:, WAVE : 2 * WAVE], st["c_prev"][:], OP.mult
                        )
                        st["m2"] = m2
                    st["sig"] = sig
                    st["m1"] = m1
                return fn

            def mk_s2(w, t):
                def fn():
                    st = state[w]
                    gates = st["gates"]
                    sig, m1 = st["sig"], st["m1"]
                    c_new = hcpool.tile([128, WAVE], f16, tag="c", name="c")
                    if t == 0:
                        nc.vector.tensor_scalar_mul(c_new[:], m1[:], 2.0)
                        st["h_all"] = opool.tile(
                            [128, T * WAVE], f16, tag="hall", name="hall"
                        )
                    else:
                        nc.vector.scalar_tensor_tensor(
                            c_new[:], m1[:], 2.0, st["m2"][:], OP.mult, OP.add
                        )
                    tanhc = apool.tile([128, WAVE], f16, tag="tanhc", name="tanhc")
                    nc.scalar.activation(tanhc[:], c_new[:], AF.Tanh)
                    h_new = st["h_all"][:, ts(t, WAVE)]
                    nc.vector.tensor_tensor(
                        h_new[:], sig[:, 2 * WAVE : 3 * WAVE], tanhc[:], OP.mult
                    )
                    if t < T - 1:
                        dh = apool.tile([128, WAVE], f16, tag="dh", name="dh")
                        nc.gpsimd.tensor_tensor(
                            dh[:], h_new[:], st["h_prev"][:], OP.subtract
                        )
                        for g in range(4):
                            nc.tensor.matmul(
                                gates[:, ts(g, WAVE)], whht_s[:, ts(g, 128)], dh[:],
                                start=False, stop=(t == T - 2),
                                skip_group_check=True,
                            )
                            if t == 0:
                                nc.tensor.matmul(
                                    gates[:, ts(g, WAVE)], wes_s[:, ts(g, 128)],
                                    st["k3rhs"][:],
                                    start=False, stop=False, skip_group_check=True,
                                )
                    st["h_prev"] = h_new
                    st["c_prev"] = c_new
                return fn

            def mk_rel(w):
                def fn():
                    st = state[w]
                    for q in range(T // 2):
                        psr2 = scrpool.tile(
                            [2, 2 * WAVE], f32, tag="scratch", name="psr2"
                        )
                        nc.tensor.matmul(
                            psr2[:], whpt_s[:], st["h_all"][:, ts(q, 2 * WAVE)],
                            start=True, stop=True,
                        )
                        nc.vector.tensor_copy(
                            st["predsb"][:, ts(q, 2 * WAVE)], psr2[:]
                        )
                    nc.sync.dma_start(
                        pred_v[:, :, ts(w, WAVE)],
                        st["predsb"].rearrange("j (t b) -> j t b", b=WAVE),
                    )
                return fn

            # dense ramp: waves 0-2 own the 3 gate tiles so they can start
            # close together; from wave 3 on, +5 keeps starts[w]-starts[w-3]
            # >= 15 = tile occupancy 14 + 1 round slack (0 slack deadlocks
            # the scheduler).
            starts = [0, 3, 6]
            for w in range(3, NW):
                starts.append(starts[w - 1] + 5)
            for w in range(NW):
                s0 = starts[w]  # init rounds s0, s0+1; steps at s0+2..s0+13
                events.append((s0, 1, mk_init1(w)))
                events.append((s0 + 1, 1, mk_init2(w)))
                for t in range(T):
                    r = s0 + 2 + t
                    events.append((r, 2 + (T - t), mk_s1(w, t)))
                    events.append((r, 20 + (T - t), mk_s2(w, t)))
                events.append((s0 + 14, 2, mk_rel(w)))
            for p in range(NP):
                # 9 mlp filler units during the 9 rounds before init1(2p)
                base = starts[2 * p] - 10
                for j in range(8):
                    events.append((base + j, 15, mk_mlp1(p, j)))
                events.append((base + 8, 15, mk_mlp2(p)))

            for _, _, fn in sorted(events, key=lambda e: (e[0], e[1])):
                fn()

    nc.compile()
    return nc


def _prep(inputs):
    """Host-side weight folding + layout prep. Returns per-core input maps."""
    f = np.float64
    W_ih = np.asarray(inputs["W_ih"], f)
    W_hh = np.asarray(inputs["W_hh"], f)
    b_ih = np.asarray(inputs["b_ih"], f)
    b_hh = np.asarray(inputs["b_hh"], f)
    W1 = np.asarray(inputs["W1"], f)
    b1 = np.asarray(inputs["b1"], f)
    W2 = np.asarray(inputs["W2"], f)
    b2 = np.asarray(inputs["b2"], f)
    W_sp = np.asarray(inputs["W_sp"], f)
    b_sp = np.asarray(inputs["b_sp"], f)
    W_hp = np.asarray(inputs["W_hp"], f)
    b_hp = np.asarray(inputs["b_hp"], f)

    W_zx = W_ih[:, :ZX]
    W_emb = W_ih[:, ZX:]
    W_es = W_emb @ W_sp                       # [4H, 2]
    W_hh_f = W_hh + W_es @ W_hp               # [4H, H]
    bias1 = b_ih + b_hh + W_emb @ b_sp + W_es @ b_hp

    # reorder pytorch gates (i, f, g, o) -> (i, f, o, g)
    perm = np.r_[0:H, H : 2 * H, 3 * H : 4 * H, 2 * H : 3 * H]
    W_zx = W_zx[perm]
    W_hh_f = W_hh_f[perm]
    W_es = W_es[perm]
    bias1 = bias1[perm]
    # double the g-gate block: its bank then holds 2*g_pre, so
    # tanh(g) = 2*sigmoid(2*g_pre) - 1 comes out of the one big sigmoid
    dbl = np.ones((G4, 1))
    dbl[3 * H :] = 2.0
    W_zx = W_zx * dbl
    W_hh_f = W_hh_f * dbl
    W_es = W_es * dbl
    bias1 = bias1 * dbl[:, 0]

    def kxm(Wt, kp):  # [K, M] -> [128, K/128, M] fp16, zero-padded to kp rows
        K, M = Wt.shape
        out = np.zeros((kp, M), f)
        out[:K] = Wt
        return np.ascontiguousarray(
            out.reshape(kp // 128, 128, M).transpose(1, 0, 2)
        ).astype(np.float16)

    consts = {
        "w1t": kxm(W1.T, KP),
        "wzxt": kxm(W_zx.T, KP),
        "w2t": kxm(W2.T, MLP),
        "whht": np.ascontiguousarray(W_hh_f.T).astype(np.float16),
        "whpt": np.ascontiguousarray(W_hp.T).astype(np.float16),
        "k3": np.ascontiguousarray(-W_es.T).astype(np.float16),
        "wes": np.ascontiguousarray(W_es.T).astype(np.float16),
        # bank-open bias: bias2[r, bk*128+m] = bias1[(2*bk + r)*128 + m]
        "bias2": np.ascontiguousarray(
            bias1.reshape(4, 128).reshape(2, 2, 128).transpose(1, 0, 2).reshape(2, 256)
        ).astype(np.float16),
        # 0/1 indicator selecting which half-bank gets which bias row
        "ind": np.kron(np.eye(2), np.ones((1, WAVE))).astype(np.float16),
        "b1": np.ascontiguousarray(b1.reshape(8, 128).T).astype(np.float32),
        "b2": b2.reshape(128, 1).astype(np.float32),
        "bhp": b_hp.reshape(2, 1).astype(np.float32),
    }

    enc = np.asarray(inputs["enc_h_feat"], np.float32)
    z = np.asarray(inputs["z"], np.float32)
    lpr = np.asarray(inputs["last_pos_rel"], np.float32)
    zxT = np.zeros((KP, B), np.float16)
    zxT[:MLP] = enc.T
    zxT[MLP:ZX] = z.T
    lprT = np.ascontiguousarray(lpr.T).astype(np.float16)

    in_maps = []
    for c in range(NCORES):
        s = slice(c * BC, (c + 1) * BC)
        m = dict(consts)
        m["zxT"] = np.ascontiguousarray(zxT[:, s])
        m["lprT"] = np.ascontiguousarray(lprT[:, s])
        in_maps.append(m)
    return in_maps


def run(inputs, trace=False):
    from concourse.bass_utils import run_bass_kernel_spmd

    if "nc" not in _cache:
        _cache["nc"] = _build_nc()
    in_maps = _prep(inputs)
    res = run_bass_kernel_spmd(
        _cache["nc"], in_maps, core_ids=list(range(NCORES)), trace=trace
    )
    pred = np.concatenate([r["pred"] for r in res.results], axis=2)  # [T, 2, B]
    out = pred.transpose(0, 2, 1) + np.asarray(inputs["b_hp"], np.float32)[None, None, :]
    return np.ascontiguousarray(out), res


def kernel(**inputs) -> np.ndarray:
    out, _ = run(inputs, trace=False)
    return out

